# revision 6
# baseline (speedup 1.0000x reference)
"""Multi-head causal attention (RoPE) on 8 TRN2 NeuronCores.

Sharding: tensor-parallel over heads. Each core computes 2 of the 16 heads:
column-parallel q/k/v projections, local attention, then an AllGather of the
(transposed) attention outputs and a column-parallel o-projection (each core
produces a 128-wide slice of the output feature dim).

Layout strategy: activations live transposed on-chip ([dim, token]), so every
matmul contracts over the partition axis with zero on-chip transposes of x.
Scores are computed transposed ([tk, tq]); softmax normalization uses a fused
ones-column in the PV matmul (denominator lands in PSUM row 64) and a
per-partition reciprocal-scale, after PV is computed in [tq, dh] form.
RoPE uses the interleaved-pair identity q' = q*C + swap(q)*S', with the pair
swap done by the DVE stream-shuffle (partition pair swap within 32-groups).
"""

import sys

for _p in ("/opt/trn_rl_repo",):
    if _p not in sys.path:
        sys.path.insert(0, _p)

import numpy as np
import ml_dtypes

import concourse.bass as bass
import concourse.mybir as mybir
import concourse.tile as tile
from concourse import bacc
from concourse.bass_utils import run_bass_kernel_spmd
from concourse.masks import make_identity

# Problem constants (nn_MultiHeadAttention: x [4,1024,1024], 16 heads)
B, T, D = 4, 1024, 1024
H, DH = 16, 64
NCORES = 8
HPC = H // NCORES          # heads per core = 2
DPC = HPC * DH             # head-dims per core = 128
BT = B * T                 # 4096 tokens
CT = D // 128              # 8 contraction tiles of 128
NCHUNK = BT // 512         # 8 free-dim chunks of 512
TPB = T // 128             # 8 key/query 128-tiles per batch row
ROPE_BASE = 10000.0

F32 = mybir.dt.float32
BF16 = mybir.dt.bfloat16
AF = mybir.ActivationFunctionType
ALU = mybir.AluOpType

SWAP_MASK = [i ^ 1 for i in range(32)]  # pair swap within each 32-partition group

_compiled = {}


def _build_nc():
    nc = bacc.Bacc(None, target_bir_lowering=False, debug=False)

    xT = nc.declare_dram_parameter("xT", [D, BT], BF16, isOutput=False)
    wqT = nc.declare_dram_parameter("wqT", [D, DPC], BF16, isOutput=False)
    wkT = nc.declare_dram_parameter("wkT", [D, DPC], BF16, isOutput=False)
    wvT = nc.declare_dram_parameter("wvT", [D, DPC], BF16, isOutput=False)
    woT = nc.declare_dram_parameter("woT", [D, DPC], BF16, isOutput=False)
    cosb = nc.declare_dram_parameter("cosb", [DPC, T], F32, isOutput=False)
    sinb = nc.declare_dram_parameter("sinb", [DPC, T], F32, isOutput=False)
    triu = nc.declare_dram_parameter("triu", [128, 128], BF16, isOutput=False)
    yT = nc.declare_dram_parameter("yT", [DPC, BT], F32, isOutput=True)

    with tile.TileContext(nc) as tc:
        import contextlib

        ctx = contextlib.ExitStack()
        with ctx:
            dram = ctx.enter_context(tc.tile_pool(name="dram", bufs=1, space="DRAM"))
            ag_in = dram.tile([DPC, BT], BF16)
            ag_out = dram.tile([D, BT], BF16, addr_space="Shared")

            consts = ctx.enter_context(tc.tile_pool(name="consts", bufs=1))
            ident = consts.tile([128, 128], BF16)
            make_identity(nc, ident[:])
            cos_sb = consts.tile([DPC, T], F32)
            sin_sb = consts.tile([DPC, T], F32)
            triu_sb = consts.tile([128, 128], BF16)
            nc.sync.dma_start(cos_sb[:], cosb[:])
            nc.sync.dma_start(sin_sb[:], sinb[:])
            nc.sync.dma_start(triu_sb[:], triu[:])

            # Weights: [1024, 128] DRAM -> [128, 8*128] SBUF (ct-blocks on free axis)
            wq_sb = consts.tile([128, CT * DPC], BF16)
            wk_sb = consts.tile([128, CT * DPC], BF16)
            wv_sb = consts.tile([128, CT * DPC], BF16)
            wo_sb = consts.tile([128, CT * DPC], BF16)
            for w_sb, w_dr in ((wq_sb, wqT), (wk_sb, wkT), (wv_sb, wvT), (wo_sb, woT)):
                for ct in range(CT):
                    nc.sync.dma_start(
                        w_sb[:, ct * DPC:(ct + 1) * DPC],
                        w_dr[ct * 128:(ct + 1) * 128, :],
                    )

            pers = ctx.enter_context(tc.tile_pool(name="pers", bufs=1))
            qT_sb = pers.tile([128, BT], BF16)
            kT_sb = pers.tile([128, BT], BF16)
            vT_sb = pers.tile([128, BT], BF16)
            aoT_sb = pers.tile([128, BT], BF16)

            # ---------------- Phase 1: QKV projections + RoPE ----------------
            with contextlib.ExitStack() as p1:
                xpool = p1.enter_context(tc.tile_pool(name="xT", bufs=1))
                xts = []
                for ct in range(CT):
                    xt = xpool.tile([128, BT], BF16, tag=f"x{ct}")
                    nc.sync.dma_start(xt[:], xT[ct * 128:(ct + 1) * 128, :])
                    xts.append(xt)

                pps = p1.enter_context(
                    tc.tile_pool(name="qkv_psum", bufs=2, space="PSUM"))
                rtp = p1.enter_context(tc.tile_pool(name="rope_tmp", bufs=2))

                for ch in range(NCHUNK):
                    sl = slice(ch * 512, ch * 512 + 512)
                    tsl = slice((ch % 2) * 512, (ch % 2) * 512 + 512)
                    pq = pps.tile([128, 512], F32, tag="pq")
                    pk = pps.tile([128, 512], F32, tag="pk")
                    pv = pps.tile([128, 512], F32, tag="pv")
                    for ct in range(CT):
                        st, sp = (ct == 0), (ct == CT - 1)
                        wsl = slice(ct * DPC, (ct + 1) * DPC)
                        nc.tensor.matmul(pq[:], wq_sb[:, wsl], xts[ct][:, sl],
                                         start=st, stop=sp)
                        nc.tensor.matmul(pk[:], wk_sb[:, wsl], xts[ct][:, sl],
                                         start=st, stop=sp)
                        nc.tensor.matmul(pv[:], wv_sb[:, wsl], xts[ct][:, sl],
                                         start=st, stop=sp)
                    nc.scalar.copy(vT_sb[:, sl], pv[:])
                    for psrc, dst in ((pq, qT_sb), (pk, kT_sb)):
                        sw = rtp.tile([128, 512], F32, tag="sw")
                        m1 = rtp.tile([128, 512], F32, tag="m1")
                        m2 = rtp.tile([128, 512], F32, tag="m2")
                        nc.vector.stream_shuffle(sw[:], psrc[:], SWAP_MASK)
                        nc.vector.tensor_tensor(m1[:], psrc[:], cos_sb[:, tsl], ALU.mult)
                        nc.vector.tensor_tensor(m2[:], sw[:], sin_sb[:, tsl], ALU.mult)
                        nc.vector.tensor_tensor(dst[:, sl], m1[:], m2[:], ALU.add)

            # ---------------- Phase 2: v transpose into [tk, dh] tiles -------
            v_tiles = {}
            with contextlib.ExitStack() as p2:
                vpool = ctx.enter_context(tc.tile_pool(name="v_sb", bufs=1))
                tpp = p2.enter_context(
                    tc.tile_pool(name="vt_psum", bufs=2, space="PSUM"))
                for b in range(B):
                    for kt in range(TPB):
                        c0 = b * T + kt * 128
                        pt = tpp.tile([128, 128], BF16, tag="pt")
                        nc.tensor.transpose(pt[:], vT_sb[:, c0:c0 + 128], ident[:])
                        vsb = vpool.tile([128, 130], BF16, tag=f"v{b}_{kt}")
                        nc.scalar.copy(vsb[:, 0:64], pt[:, 0:64])
                        nc.scalar.copy(vsb[:, 65:129], pt[:, 64:128])
                        nc.vector.memset(vsb[:, 64:65], 1.0)
                        nc.vector.memset(vsb[:, 129:130], 1.0)
                        v_tiles[(b, kt)] = vsb

            # ---------------- Phase 3: attention per batch row ---------------
            with contextlib.ExitStack() as p3:
                epool = p3.enter_context(tc.tile_pool(name="E", bufs=1))
                spsum = p3.enter_context(
                    tc.tile_pool(name="s_psum", bufs=2, space="PSUM"))
                opsum = p3.enter_context(
                    tc.tile_pool(name="o_psum", bufs=2, space="PSUM"))
                tpsum = p3.enter_context(
                    tc.tile_pool(name="t_psum", bufs=2, space="PSUM"))
                small = p3.enter_context(tc.tile_pool(name="small", bufs=4))
                aopool = p3.enter_context(tc.tile_pool(name="ao", bufs=1))

                for b in range(B):
                    e_tiles = {}
                    for h in range(HPC):
                        for kt in range(TPB):
                            e_tiles[(h, kt)] = epool.tile(
                                [128, T], BF16, tag=f"e{h}_{kt}",
                                name=f"e_b{b}_{h}_{kt}")
                    # scores (transposed) + exp + causal mask
                    for kt in range(TPB):
                        for qc in range(2):
                            if (qc + 1) * 512 <= kt * 128:
                                continue  # chunk entirely in the causal past
                            ls = max(0, kt * 128 - qc * 512)
                            for h in range(HPC):
                                hsl = slice(h * 64, (h + 1) * 64)
                                ps = spsum.tile([128, 512], F32, tag=f"s{h}")
                                nc.tensor.matmul(
                                    ps[:],
                                    kT_sb[hsl, b * T + kt * 128:b * T + kt * 128 + 128],
                                    qT_sb[hsl, b * T + qc * 512:b * T + (qc + 1) * 512],
                                    start=True, stop=True,
                                )
                                nc.scalar.activation(
                                    e_tiles[(h, kt)][:, qc * 512 + ls:(qc + 1) * 512],
                                    ps[:, ls:512],
                                    AF.Exp, scale=float(DH) ** -0.5,
                                )
                        # causal triangular mask on the diagonal 128-block
                        dsl = slice(kt * 128, kt * 128 + 128)
                        for h in range(HPC):
                            nc.vector.tensor_tensor(
                                e_tiles[(h, kt)][:, dsl],
                                e_tiles[(h, kt)][:, dsl],
                                triu_sb[:], ALU.mult,
                            )
                    # PV + normalize
                    ao_b = []
                    for qt in range(TPB):
                        ao_b.append(aopool.tile([128, 128], BF16, tag=f"ao{qt}",
                                                name=f"ao_b{b}_{qt}"))
                    for h in range(HPC):
                        for qt in range(TPB):
                            po = opsum.tile([128, 65], F32, tag="po")
                            for kt in range(qt + 1):
                                nc.tensor.matmul(
                                    po[:],
                                    e_tiles[(h, kt)][:, qt * 128:qt * 128 + 128],
                                    v_tiles[(b, kt)][:, h * 65:h * 65 + 65],
                                    start=(kt == 0), stop=(kt == qt),
                                )
                            r = small.tile([128, 1], F32, tag="r")
                            nc.vector.reciprocal(r[:], po[:, 64:65])
                            nc.vector.tensor_scalar(
                                out=ao_b[qt][:, h * 64:(h + 1) * 64],
                                in0=po[:, 0:64],
                                scalar1=r[:], scalar2=None, op0=ALU.mult,
                            )
                    # transpose [tq, c] -> [c, tq] into aoT
                    for qt in range(TPB):
                        pt2 = tpsum.tile([128, 128], BF16, tag="pt2")
                        nc.tensor.transpose(pt2[:], ao_b[qt][:], ident[:])
                        nc.scalar.copy(
                            aoT_sb[:, b * T + qt * 128:b * T + qt * 128 + 128], pt2[:])

            # ---------------- Phase 4: AllGather + o-projection --------------
            nc.sync.dma_start(ag_in[:], aoT_sb[:])
            nc.gpsimd.collective_compute(
                "AllGather", ALU.bypass,
                replica_groups=[list(range(NCORES))],
                ins=[ag_in[:]], outs=[ag_out[:]],
            )
            with contextlib.ExitStack() as p4:
                aof_pool = p4.enter_context(tc.tile_pool(name="aof", bufs=3))
                ypsum = p4.enter_context(
                    tc.tile_pool(name="y_psum", bufs=1, space="PSUM"))
                yout = p4.enter_context(tc.tile_pool(name="yout", bufs=3))
                pys = [ypsum.tile([128, 512], F32, tag=f"y{n}", name=f"py{n}")
                       for n in range(NCHUNK)]
                for ct in range(CT):
                    aof = aof_pool.tile([128, BT], BF16, tag="aof")
                    nc.sync.dma_start(aof[:], ag_out[ct * 128:(ct + 1) * 128, :])
                    for n in range(NCHUNK):
                        nc.tensor.matmul(
                            pys[n][:],
                            wo_sb[:, ct * DPC:(ct + 1) * DPC],
                            aof[:, n * 512:(n + 1) * 512],
                            start=(ct == 0), stop=(ct == CT - 1),
                        )
                for n in range(NCHUNK):
                    yo = yout.tile([128, 512], F32, tag="yo")
                    nc.scalar.copy(yo[:], pys[n][:])
                    nc.sync.dma_start(yT[:, n * 512:(n + 1) * 512], yo[:])

    nc.compile()
    return nc


def _host_inputs(x, Wq, Wk, Wv, Wo):
    bf16 = ml_dtypes.bfloat16
    x2 = np.asarray(x, dtype=np.float32).reshape(BT, D)
    xT = np.ascontiguousarray(x2.T).astype(bf16)

    half = DH // 2
    inv_freq = 1.0 / (ROPE_BASE ** (np.arange(0, DH, 2, dtype=np.float32) / DH))
    tpos = np.arange(T, dtype=np.float32)
    freqs = np.outer(tpos, inv_freq).astype(np.float32)   # [T, 32]
    cos = np.cos(freqs).astype(np.float32)
    sin = np.sin(freqs).astype(np.float32)
    pidx = (np.arange(DPC) % DH) // 2
    cosb = np.ascontiguousarray(cos.T[pidx, :]).astype(np.float32)  # [128, T]
    sign = np.where(np.arange(DPC) % 2 == 0, -1.0, 1.0).astype(np.float32)
    sinb = np.ascontiguousarray(sin.T[pidx, :] * sign[:, None]).astype(np.float32)

    triu = np.triu(np.ones((128, 128), np.float32)).astype(bf16)

    in_maps = []
    for i in range(NCORES):
        sl = slice(i * DPC, (i + 1) * DPC)
        m = {
            "xT": xT,
            "wqT": np.ascontiguousarray(np.asarray(Wq, np.float32)[sl, :].T).astype(bf16),
            "wkT": np.ascontiguousarray(np.asarray(Wk, np.float32)[sl, :].T).astype(bf16),
            "wvT": np.ascontiguousarray(np.asarray(Wv, np.float32)[sl, :].T).astype(bf16),
            "woT": np.ascontiguousarray(np.asarray(Wo, np.float32)[sl, :].T).astype(bf16),
            "cosb": cosb,
            "sinb": sinb,
            "triu": triu,
        }
        in_maps.append(m)
    return in_maps


def kernel(x, Wq, Wk, Wv, Wo, _trace=False):
    if "nc" not in _compiled:
        _compiled["nc"] = _build_nc()
    nc = _compiled["nc"]
    in_maps = _host_inputs(x, Wq, Wk, Wv, Wo)
    res = run_bass_kernel_spmd(nc, in_maps, list(range(NCORES)), trace=_trace)
    _compiled["last_result"] = res
    yT_full = np.concatenate([res.results[i]["yT"] for i in range(NCORES)], axis=0)
    y = np.ascontiguousarray(yT_full.T).reshape(B, T, D).astype(np.float32)
    return y
